# revision 38
# baseline (speedup 1.0000x reference)
"""Trainium2 Bass kernel for nn_GTShapelet (GIN stack + CLS-query MHA).

Self-contained: builds the Bass/Tile program, shards inputs across 8
NeuronCores (data-parallel over destination-node ranges; graphs 4c..4c+3
on core c), runs via run_bass_kernel_spmd, and reassembles the full
[32, 128] output.

Key algorithmic points:
  - Only y[:, -1, :] (the CLS token) is returned by the reference, so the
    attention needs just one query per graph, and that query is
    h-independent: per-head q^T Wk folds to a [128, 4] matrix on the host
    so scores come straight from h3 via one matmul per 128-node tile.
  - Layer 1 folds embed_table @ W1 into a 1024x256 table T1; its
    edge-weighted scatter-sum becomes a dense [1024, 4096] x [1024, 256]
    PE matmul against a host-built weighted count matrix ct (fp8).
  - Layers 2/3 gather per-edge source rows with dma_gather, but from
    *post-weight* tables (g1 = h1 @ W2 in fp8, p2 = h2 @ W3 in bf16),
    distributing the GIN linear layer through the segment-sum so no
    node-matmul remains after aggregation.  All aggregation matmuls are
    "flipped": the gathered batch is lhsT and the DVE-built selection
    matrix S is rhs, so each 128-edge batch streams only 64 destination
    columns and the result lands feature-major (no transposes).
  - Between layers the 8 cores exchange tables with AllGathers, chunked
    in two halves so the first half overlaps the producing phase (gather
    indices are host-remapped to the half-concatenated row layout).
  - Chunk-overflow ("tail") edges are compacted per 512-dst segment into
    one 512-slot gather call with segment-local selection matrices, since
    the dominant real cost is per gather index (~7ns/idx on this axon
    backend, measured; PE/DVE/Act work and collectives pipeline to ~free).
  - `repeat=K` builds a program that runs the whole pipeline K times;
    test.py times K=13 vs K=1 executions to cancel the multi-ms host
    staging cost and recover the per-iteration execution time.
"""

import sys

if "/opt/trn_rl_repo" not in sys.path:
    sys.path.insert(0, "/opt/trn_rl_repo")

import numpy as np
import ml_dtypes  # noqa: F401  (np 'bfloat16' dtype registration)

# ---- problem constants (hardcoded per spec) ----
B, N, E, D = 32, 1024, 524288, 128
H, HD = 4, 32
F2 = 2 * D  # 256
NCORES = 8
NPC = B * N // NCORES          # 4096 nodes per core
GPC = B // NCORES              # 4 graphs per core
CHW = 64                       # dst-chunk width (nodes)
NCH = NPC // CHW               # 64 chunks per core
NB = 9                         # batches per chunk: 8 main + 1 tail
ACAP = 1024                    # main gather slots per chunk (one call)
TAIL = 128                     # tail slots per chunk
CHCAP = ACAP + TAIL            # 1152 edge capacity per chunk
SEG = 8                        # chunks per segment
NSEG = NCH // SEG              # 8 segments per core
SEGW = SEG * CHW               # 512 dst per segment
SEGSLOTS = SEG * ACAP + SEG * TAIL   # 9216: [8x1024 main | 8x128 tails]
CAP = NSEG * SEGSLOTS          # 73728 slots per core
NBAT = SEG * NB                # 72 batches per segment (64 main + 8 tail)
BF16 = np.dtype('bfloat16')
F8 = np.dtype(ml_dtypes.float8_e4m3)

_prog_cache = {}


def _build_program(variant="hw", phases=4, repeat=1):
    import os as _os
    nog = int(_os.environ.get("GK_NOG", "0"))   # timing exp: skip gathers
    qn = int(_os.environ.get("GK_QN", "1"))     # swdge queues
    spl = int(_os.environ.get("GK_SPLIT", "0"))  # 512-idx gather calls
    l3f8 = int(_os.environ.get("GK_L3F8", "0"))  # p2 table fp8 256-elem rows
    nold = int(_os.environ.get("GK_NOLOAD", "0"))  # static counts, -1 term
    spk = int(_os.environ.get("GK_SP", "0"))     # single_packet flag
    scr = int(_os.environ.get("GK_SCR", "32768"))  # dma scratch bytes
    key = (variant, phases, repeat, nog, qn, spl, l3f8, nold, spk, scr)
    if key in _prog_cache:
        return _prog_cache[key]
    import concourse.bacc as bacc
    import concourse.tile as tile
    import concourse.mybir as mybir
    from concourse.library_config import mlp

    dt = mybir.dt
    AF = mybir.ActivationFunctionType
    OP = mybir.AluOpType

    nc = bacc.Bacc("TRN2", target_bir_lowering=False, debug=False,
                   num_devices=(1 if variant == "sim1" else NCORES),
                   dynamic_dma_scratch_size=scr, num_swdge_queues=qn)

    def din(name, shape, dtype):
        return nc.dram_tensor(name, shape, dtype, kind="ExternalInput")

    t1 = din("t1", [N, F2], dt.bfloat16)
    h0w1T = din("h0w1T", [128, 2, NPC], dt.bfloat16)
    ct = din("ct", [128, N // 128, NPC], dt.float8e4)
    idx12 = din("idx12", [128, CAP // 16], dt.int16)
    cnts = din("cnts", [1, NCH + NSEG], dt.int32)
    dstl = din("dstl", [128, NSEG * NBAT], dt.bfloat16)
    eww = din("eww", [128, NSEG * NBAT], dt.bfloat16)
    iota64 = din("iota64", [128, CHW], dt.bfloat16)
    iota512 = din("iota512", [128, SEGW], dt.float32)
    dstl3 = din("dstl3", [128, NSEG * 4], dt.float32)
    eww3 = din("eww3", [128, NSEG * 4], dt.bfloat16)
    i128 = din("i128", [128, 128], dt.bfloat16)
    i128f = din("i128f", [128, 128], dt.float32)
    w2t = din("w2t", [128, 4 * 128], dt.bfloat16)
    b2 = din("b2", [128, 2], dt.float32)
    w3t = din("w3t", [128, 2 * 128], dt.bfloat16)
    b3 = din("b3", [128, 1], dt.float32)
    b1 = din("b1", [128, 2], dt.float32)
    wkq = din("wkq", [128, 4], dt.bfloat16)
    wv = din("wv", [128, 128], dt.bfloat16)
    bv = din("bv", [128, 1], dt.float32)
    ones1 = din("ones1", [128, 1], dt.bfloat16)
    map16 = din("map16", [128, 16], dt.bfloat16)
    ecls16 = din("ecls16", [16, 1], dt.float32)
    vc4 = din("vc4", [4, 128], dt.bfloat16)
    e4 = din("e4", [4, 4], dt.bfloat16)
    msel = din("msel", [128, 4], dt.float32)
    hexp = din("hexp", [16, 128], dt.float32)
    gmask = din("gmask", [16, 4], dt.float32)
    eps = din("eps", [4, 1], dt.float32)
    ynb = din("ynb", [128, 1], dt.float32)
    wo = din("wo", [128, 128], dt.bfloat16)
    lng = din("lng", [4, 128], dt.float32)
    lnb = din("lnb", [4, 128], dt.float32)
    y_out = nc.dram_tensor("y", [GPC, D], dt.float32, kind="ExternalOutput")

    with tile.TileContext(nc) as tc:
        nc.gpsimd.load_library(mlp)
        with tc.tile_pool(name="const", bufs=1) as cp, \
             tc.tile_pool(name="res", bufs=1) as rp, \
             tc.tile_pool(name="work", bufs=1) as wp, \
             tc.tile_pool(name="dram", bufs=2, space="DRAM") as dram:

            def cload(ap, shape, dtype, pool=cp):
                t = pool.tile(shape, dtype, name=f"c_{ap.name}")
                nc.sync.dma_start(out=t[:], in_=ap[:])
                return t

            # L0-critical loads issue first so segment 0 starts immediately
            i128_t = cload(i128, [128, 128], dt.bfloat16)
            b1_t = cload(b1, [128, 2], dt.float32)
            w2t_t = cload(w2t, [128, 4 * 128], dt.bfloat16)
            h0T_t = cload(h0w1T, [128, 2, NPC], dt.bfloat16)

            idx_t = cload(idx12, [128, CAP // 16], dt.int16)
            cnts_t = cload(cnts, [1, NCH + NSEG], dt.int32)
            dstl_t = cload(dstl, [128, NSEG * NBAT], dt.bfloat16)
            eww_t = cload(eww, [128, NSEG * NBAT], dt.bfloat16)
            iota_t = cload(iota64, [128, CHW], dt.bfloat16)
            iota512_t = cload(iota512, [128, SEGW], dt.float32)
            dstl3_t = cload(dstl3, [128, NSEG * 4], dt.float32)
            eww3_t = cload(eww3, [128, NSEG * 4], dt.bfloat16)
            i128f_t = cload(i128f, [128, 128], dt.float32)
            b2_t = cload(b2, [128, 2], dt.float32)
            w3t_t = cload(w3t, [128, 2 * 128], dt.bfloat16)
            b3_t = cload(b3, [128, 1], dt.float32)
            wkq_t = cload(wkq, [128, 4], dt.bfloat16)
            wv_t = cload(wv, [128, 128], dt.bfloat16)
            bv_t = cload(bv, [128, 1], dt.float32)
            ones1_t = cload(ones1, [128, 1], dt.bfloat16)
            map16_t = cload(map16, [128, 16], dt.bfloat16)
            ecls16_t = cload(ecls16, [16, 1], dt.float32)
            vc4_t = cload(vc4, [4, 128], dt.bfloat16)
            e4_t = cload(e4, [4, 4], dt.bfloat16)
            msel_t = cload(msel, [128, 4], dt.float32)
            hexp_t = cload(hexp, [16, 128], dt.float32)
            gmask_t = cload(gmask, [16, 4], dt.float32)
            eps_t = cload(eps, [4, 1], dt.float32)
            ynb_t = cload(ynb, [128, 1], dt.float32)
            wo_t = cload(wo, [128, 128], dt.bfloat16)
            lng_t = cload(lng, [4, 128], dt.float32)
            lnb_t = cload(lnb, [4, 128], dt.float32)

            # persistent feature-major layer state
            g1T = rp.tile([128, 2, NPC], dt.bfloat16, name="g1T")
            hT2 = rp.tile([128, 2, NPC], dt.bfloat16, name="hT2")
            p2T = rp.tile([128, NPC], dt.bfloat16, name="p2T")
            hT3 = rp.tile([128, NPC], dt.bfloat16, name="hT3")
            # gather buffers: 6-deep chunk rotation + per-seg tail bufs;
            # L2 gathers fp8 (table g1 is fp8), L3 gathers bf16.
            NROT = 6
            xb2 = [wp.tile([128, 8, F2], dt.float8e4, tag=f"x2_{i}",
                           name=f"xb2_{i}") for i in range(NROT)]
            tb2 = [wp.tile([128, 4, F2], dt.float8e4, tag=f"t2_{i}",
                           name=f"tb2_{i}") for i in range(2)]
            e3 = F2 if l3f8 else D
            dt3 = dt.float8e4 if l3f8 else dt.bfloat16
            xb3 = [wp.tile([128, 8, e3], dt3, tag=f"x3_{i}",
                           name=f"xb3_{i}") for i in range(NROT)]
            tb3 = [wp.tile([128, 4, e3], dt3, tag=f"t3_{i}",
                           name=f"tb3_{i}") for i in range(2)]
            for t_ in xb2 + tb2 + xb3 + tb3:
                nc.gpsimd.memset(t_[:], 0)
            sseg = [wp.tile([128, NBAT * CHW], dt.bfloat16,
                            tag=f"sseg{i}", name=f"sseg{i}")
                    for i in range(2)]
            sseg3 = [wp.tile([128, 4 * SEGW], dt.bfloat16,
                             tag=f"ss3_{i}", name=f"sseg3_{i}")
                     for i in range(2)]
            stg1 = [wp.tile([128, 4, F2], dt.float8e4, tag=f"st1{i}",
                            name=f"stg1{i}") for i in range(2)]
            stg2 = [wp.tile([128, 4, e3], dt3, tag=f"st2{i}",
                            name=f"stg2{i}") for i in range(2)]
            for t_ in stg2:
                nc.gpsimd.memset(t_[:], 0)
            expt = rp.tile([128, 128], dt.bfloat16, name="expt")
            scsb = rp.tile([128, 128], dt.float32, name="scsb")
            vnm = rp.tile([128, NPC // 128, 128], dt.bfloat16, name="vnm")

            agin1 = dram.tile([NPC, F2], dt.float8e4, tag="agin1")
            agout1 = dram.tile([B * N, F2], dt.float8e4, tag="agout1")
            agin2 = dram.tile([NPC, e3], dt3, tag="agin2")
            agout2 = dram.tile([B * N, e3], dt3, tag="agout2")
            ag1v = agin1.rearrange("(t p) f -> p t f", p=128)
            ag2v = agin2.rearrange("(t p) f -> p t f", p=128)

            def allgather_half(agin, agout, half):
                """AllGather one half of the per-core table.  Output rows
                of half h: [16384*h + c*2048, ...) for core c (the host
                remaps gather indices to match)."""
                hnpc = NPC // 2
                ain = agin[half * hnpc:(half + 1) * hnpc, :]
                aout = agout[half * (B * N // 2):
                             (half + 1) * (B * N // 2), :]
                if variant == "sim1":
                    for cc in range(NCORES):
                        nc.sync.dma_start(
                            out=aout[cc * hnpc:(cc + 1) * hnpc, :],
                            in_=ain)
                else:
                    nc.gpsimd.collective_compute(
                        "AllGather", mybir.AluOpType.bypass,
                        replica_groups=[list(range(NCORES))],
                        ins=[ain.opt()], outs=[aout.opt()])

            def build_s(s):
                st = sseg[s % 2]
                bb0 = s * NBAT
                sv = st[:].rearrange("p (d b) -> p d b", b=NBAT)
                nc.vector.tensor_tensor(
                    out=sv,
                    in0=dstl_t[:, bb0:bb0 + NBAT].unsqueeze(1)
                        .broadcast_to([128, CHW, NBAT]),
                    in1=iota_t[:].unsqueeze(2).broadcast_to([128, CHW, NBAT]),
                    op=OP.is_equal)
                nc.vector.tensor_tensor(
                    out=sv, in0=sv,
                    in1=eww_t[:, bb0:bb0 + NBAT].unsqueeze(1)
                        .broadcast_to([128, CHW, NBAT]),
                    op=OP.mult)
                return sv

            def build_s3(s):
                st = sseg3[s % 2]
                sv3 = st[:].rearrange("p (d b) -> p d b", b=4)
                bb0 = s * 4
                nc.vector.tensor_tensor(
                    out=sv3,
                    in0=dstl3_t[:, bb0:bb0 + 4].unsqueeze(1)
                        .broadcast_to([128, SEGW, 4]),
                    in1=iota512_t[:].unsqueeze(2)
                        .broadcast_to([128, SEGW, 4]),
                    op=OP.is_equal)
                nc.vector.tensor_tensor(
                    out=sv3, in0=sv3,
                    in1=eww3_t[:, bb0:bb0 + 4].unsqueeze(1)
                        .broadcast_to([128, SEGW, 4]),
                    op=OP.mult)
                return sv3

            def seg_gathers(s, gsrc, xbufs, tailbufs, esz, layer):
                """Issue the main + tail gather calls for segment s.
                Trailing -1 indices terminate each call region, so static
                counts (nold/spl modes) stay numerically correct."""
                if nog & layer:
                    return
                if nold or spl:
                    cregs = [1024] * SEG
                    cregt = 512
                else:
                    cregs = [nc.gpsimd.value_load(
                        cnts_t[0:1, s * SEG + kk:s * SEG + kk + 1])
                        for kk in range(SEG)]
                    cregt = nc.gpsimd.value_load(
                        cnts_t[0:1, NCH + s:NCH + s + 1])
                for kk in range(SEG):
                    xb = xbufs[(s * SEG + kk) % len(xbufs)]
                    ib = (s * SEGSLOTS + kk * ACAP) // 16
                    if spl:
                        nc.gpsimd.dma_gather(
                            xb[:, 0:4, :], gsrc[:], idx_t[:, ib:ib + 32],
                            512, 512, esz, queue_num=kk % qn)
                        nc.gpsimd.dma_gather(
                            xb[:, 4:8, :], gsrc[:], idx_t[:, ib + 32:ib + 64],
                            512, 512, esz, queue_num=kk % qn)
                    else:
                        nc.gpsimd.dma_gather(
                            xb[:], gsrc[:], idx_t[:, ib:ib + 64],
                            1024, cregs[kk], esz, queue_num=kk % qn,
                            single_packet=bool(spk))
                tib = (s * SEGSLOTS + SEG * ACAP) // 16
                nc.gpsimd.dma_gather(
                    tailbufs[s % 2][:], gsrc[:], idx_t[:, tib:tib + 32],
                    512, cregt, esz, queue_num=SEG % qn,
                    single_packet=bool(spk))

            def phase_l0():
                """h1 = gelu(T1[nid] + ct @ T1 + b1); g1 = h1 @ W2;
                node-major fp8 staging -> agin1, chunked AllGather."""
                with tc.tile_pool(name="l0_ps", bufs=1, space="PSUM") as pp, \
                     tc.tile_pool(name="l0_sb", bufs=1) as lp:
                    t1sb = lp.tile([128, N // 128, F2], dt.bfloat16,
                                   name="t1sb")
                    nc.sync.dma_start(
                        out=t1sb[:],
                        in_=t1.rearrange("(kk p) f -> p kk f", p=128))
                    ctbufs = [lp.tile([128, N // 128, SEGW], dt.float8e4,
                                      tag=f"ctb{i}", name=f"ctb{i}")
                              for i in range(2)]
                    nc.sync.dma_start(out=ctbufs[0][:], in_=ct[:, :, 0:SEGW])
                    hT1 = lp.tile([128, 2, NPC], dt.bfloat16, name="hT1")
                    for s in range(NSEG):
                        sl = slice(s * SEGW, (s + 1) * SEGW)
                        ctb = ctbufs[s % 2]
                        if s > 0:
                            nc.sync.dma_start(out=ctb[:], in_=ct[:, :, sl])
                        for f in range(2):
                            ps = pp.tile([128, SEGW], dt.float32,
                                         tag=f"m{f}", bufs=2)
                            for kt in range(N // 128):
                                nc.tensor.matmul(
                                    out=ps[:],
                                    lhsT=t1sb[:, kt, f * 128:(f + 1) * 128],
                                    rhs=ctb[:, kt, :],
                                    start=(kt == 0), stop=False)
                            nc.tensor.matmul(out=ps[:], lhsT=i128_t[:],
                                             rhs=h0T_t[:, f, sl],
                                             start=False, stop=True)
                            nc.scalar.activation(hT1[:, f, sl], ps[:],
                                                 AF.Gelu,
                                                 bias=b1_t[:, f:f + 1])
                        for fo in range(2):
                            psg = pp.tile([128, SEGW], dt.float32, tag="aux",
                                          bufs=2)
                            for fi in range(2):
                                nc.tensor.matmul(
                                    out=psg[:],
                                    lhsT=w2t_t[:, (2 * fi + fo) * 128:
                                               (2 * fi + fo + 1) * 128],
                                    rhs=hT1[:, fi, sl],
                                    start=(fi == 0), stop=(fi == 1))
                            nc.vector.tensor_copy(out=g1T[:, fo, sl],
                                                  in_=psg[:])
                        st = stg1[s % 2]
                        for t4 in range(4):
                            for f in range(2):
                                tp = pp.tile([128, 128], dt.bfloat16,
                                             tag="tp", bufs=2)
                                nc.tensor.transpose(
                                    tp[:],
                                    g1T[:, f, (s * 4 + t4) * 128:
                                        (s * 4 + t4 + 1) * 128],
                                    i128_t[:])
                                nc.vector.tensor_copy(
                                    out=st[:, t4, f * 128:(f + 1) * 128],
                                    in_=tp[:])
                        nc.sync.dma_start(
                            out=ag1v[:, s * 4:(s + 1) * 4, :], in_=st[:])
                        if s == NSEG // 2 - 1:
                            allgather_half(agin1, agout1, 0)
                    allgather_half(agin1, agout1, 1)

            def phase_l2():
                """h2 = gelu(g1own + sum ew g1[src] + b2); p2 = h2 @ W3;
                bf16 staging -> agin2, chunked AllGather."""
                with tc.tile_pool(name="l2_ps", bufs=1, space="PSUM") as pp:
                    for s in range(NSEG):
                        sl = slice(s * SEGW, (s + 1) * SEGW)
                        sv = build_s(s)
                        seg_gathers(s, agout1, xb2, tb2, F2, 1)
                        ps = [pp.tile([128, SEGW], dt.float32, tag=f"m{f}",
                                      bufs=2, name=f"psm{f}")
                              for f in range(2)]
                        for f in range(2):
                            nc.tensor.matmul(out=ps[f][:], lhsT=i128_t[:],
                                             rhs=g1T[:, f, sl],
                                             start=True, stop=False)
                        for kk in range(SEG):
                            dsl = slice(kk * CHW, (kk + 1) * CHW)
                            xb = xb2[(s * SEG + kk) % NROT]
                            for bq in range(8):
                                b = kk * 8 + bq
                                for f in range(2):
                                    nc.tensor.matmul(
                                        out=ps[f][:, dsl],
                                        lhsT=xb[:, bq,
                                                f * 128:(f + 1) * 128],
                                        rhs=sv[:, :, b],
                                        start=False, stop=False)
                        sv3 = build_s3(s)
                        for tb in range(4):
                            for f in range(2):
                                nc.tensor.matmul(
                                    out=ps[f][:],
                                    lhsT=tb2[s % 2][:, tb,
                                                    f * 128:(f + 1) * 128],
                                    rhs=sv3[:, :, tb],
                                    start=False, stop=(tb == 3))
                        for f in range(2):
                            nc.scalar.activation(hT2[:, f, sl], ps[f][:],
                                                 AF.Gelu,
                                                 bias=b2_t[:, f:f + 1])
                        psp = pp.tile([128, SEGW], dt.float32, tag="aux",
                                      bufs=2)
                        for fi in range(2):
                            nc.tensor.matmul(
                                out=psp[:],
                                lhsT=w3t_t[:, fi * 128:(fi + 1) * 128],
                                rhs=hT2[:, fi, sl],
                                start=(fi == 0), stop=(fi == 1))
                        nc.vector.tensor_copy(out=p2T[:, sl], in_=psp[:])
                        st2 = stg2[s % 2]
                        for t4 in range(4):
                            tp = pp.tile([128, 128], dt.bfloat16, tag="tp",
                                         bufs=2)
                            nc.tensor.transpose(
                                tp[:],
                                p2T[:, (s * 4 + t4) * 128:
                                    (s * 4 + t4 + 1) * 128],
                                i128_t[:])
                            nc.vector.tensor_copy(out=st2[:, t4, 0:D],
                                                  in_=tp[:])
                        nc.sync.dma_start(
                            out=ag2v[:, s * 4:(s + 1) * 4, :], in_=st2[:])
                        if s == NSEG // 2 - 1:
                            allgather_half(agin2, agout2, 0)
                allgather_half(agin2, agout2, 1)

            def phase_l3():
                """h3 = gelu(p2own + sum ew p2[src] + b3); attention
                scores + values pipelined per segment."""
                with tc.tile_pool(name="l3_ps", bufs=1, space="PSUM") as pp:
                    for s in range(NSEG):
                        sl = slice(s * SEGW, (s + 1) * SEGW)
                        sv = build_s(s)
                        seg_gathers(s, agout2, xb3, tb3, e3, 2)
                        ps = pp.tile([128, SEGW], dt.float32, tag="m0",
                                     bufs=2)
                        nc.tensor.matmul(out=ps[:], lhsT=i128_t[:],
                                         rhs=p2T[:, sl],
                                         start=True, stop=False)
                        for kk in range(SEG):
                            dsl = slice(kk * CHW, (kk + 1) * CHW)
                            xb = xb3[(s * SEG + kk) % NROT]
                            for bq in range(8):
                                b = kk * 8 + bq
                                nc.tensor.matmul(out=ps[:, dsl],
                                                 lhsT=xb[:, bq, 0:D],
                                                 rhs=sv[:, :, b],
                                                 start=False, stop=False)
                        sv3 = build_s3(s)
                        for tb in range(4):
                            nc.tensor.matmul(out=ps[:],
                                             lhsT=tb3[s % 2][:, tb, 0:D],
                                             rhs=sv3[:, :, tb],
                                             start=False, stop=(tb == 3))
                        nc.scalar.activation(hT3[:, sl], ps[:], AF.Gelu,
                                             bias=b3_t[:])
                        # attention scores + values for this segment
                        psc = pp.tile([128, 16], dt.float32, tag="sc",
                                      bufs=2)
                        for t4 in range(4):
                            tsl = slice((s * 4 + t4) * 128,
                                        (s * 4 + t4 + 1) * 128)
                            nc.tensor.matmul(out=psc[:, t4 * 4:(t4 + 1) * 4],
                                             lhsT=hT3[:, tsl], rhs=wkq_t[:],
                                             start=True, stop=True)
                            psv = pp.tile([128, 128], dt.float32, tag="tp",
                                          bufs=2)
                            nc.tensor.matmul(out=psv[:], lhsT=hT3[:, tsl],
                                             rhs=wv_t[:],
                                             start=True, stop=True)
                            nc.vector.tensor_copy(out=vnm[:, s * 4 + t4, :],
                                                  in_=psv[:])
                        nc.vector.tensor_copy(
                            out=scsb[:, s * 16:(s + 1) * 16], in_=psc[:])

            def phase_attn():
                with tc.tile_pool(name="att_ps", bufs=1,
                                  space="PSUM") as ap_, \
                     tc.tile_pool(name="att_sb", bufs=1) as asb:
                    nc.scalar.activation(expt[:], scsb[:], AF.Exp)
                    pss = ap_.tile([128, 1], dt.float32, tag="pa", bufs=1)
                    nc.tensor.matmul(out=pss[:], lhsT=expt[:],
                                     rhs=ones1_t[:], start=True, stop=True)
                    s128 = asb.tile([128, 1], dt.bfloat16, tag="s128")
                    nc.vector.tensor_copy(out=s128[:], in_=pss[:])
                    psz = ap_.tile([16, 1], dt.float32, tag="pb", bufs=1)
                    nc.tensor.matmul(out=psz[:], lhsT=map16_t[:],
                                     rhs=s128[:], start=True, stop=True)
                    z16 = asb.tile([16, 1], dt.float32, tag="z16")
                    nc.vector.tensor_add(out=z16[:], in0=psz[:],
                                         in1=ecls16_t[:])
                    # rbcall[f, g] = 1 / Z[g, head(f)]
                    zm = asb.tile([16, 4], dt.float32, tag="zm")
                    nc.vector.tensor_tensor(out=zm[:], in0=gmask_t[:],
                                            in1=z16[:].broadcast_to([16, 4]),
                                            op=OP.mult)
                    psr = ap_.tile([128, 4], dt.float32, tag="pc", bufs=1)
                    nc.tensor.matmul(out=psr[:], lhsT=hexp_t[:], rhs=zm[:],
                                     start=True, stop=True)
                    rbcall = asb.tile([128, 4], dt.float32, tag="rbcall")
                    nc.vector.reciprocal(rbcall[:], psr[:])
                    ctx_all = asb.tile([128, 4], dt.bfloat16, tag="ctx_all")
                    for g in range(GPC):
                        psctx = ap_.tile([128, 4], dt.float32, tag="ctx",
                                         bufs=2)
                        for t in range(8):
                            nc.tensor.matmul(
                                out=psctx[:], lhsT=vnm[:, g * 8 + t, :],
                                rhs=expt[:, g * 32 + t * 4:
                                         g * 32 + (t + 1) * 4],
                                start=(t == 0), stop=False)
                        nc.tensor.matmul(out=psctx[:], lhsT=vc4_t[:],
                                         rhs=e4_t[:], start=False, stop=True)
                        tmp4 = asb.tile([128, 4], dt.float32, tag="tmp4",
                                        bufs=2)
                        nc.vector.tensor_tensor(out=tmp4[:], in0=psctx[:],
                                                in1=msel_t[:], op=OP.mult)
                        ctxv = asb.tile([128, 1], dt.float32, tag="ctxv",
                                        bufs=2)
                        nc.vector.reduce_sum(out=ctxv[:], in_=tmp4[:],
                                             axis=mybir.AxisListType.X)
                        nc.vector.tensor_scalar(out=ctxv[:], in0=ctxv[:],
                                                scalar1=rbcall[:, g:g + 1],
                                                scalar2=bv_t[:],
                                                op0=OP.mult, op1=OP.add)
                        nc.vector.tensor_copy(out=ctx_all[:, g:g + 1],
                                              in_=ctxv[:])
                    psao = ap_.tile([128, 4], dt.float32, tag="pa", bufs=1)
                    nc.tensor.matmul(out=psao[:], lhsT=wo_t[:],
                                     rhs=ctx_all[:], start=True, stop=True)
                    ysb = asb.tile([128, 4], dt.float32, tag="ysb")
                    nc.vector.tensor_scalar(out=ysb[:], in0=psao[:],
                                            scalar1=ynb_t[:], scalar2=None,
                                            op0=OP.add)
                    psy = ap_.tile([4, 128], dt.float32, tag="pb", bufs=1)
                    nc.tensor.matmul(out=psy[:], lhsT=ysb[:], rhs=i128f_t[:],
                                     is_transpose=True)
                    yt = asb.tile([4, 128], dt.float32, tag="yt")
                    nc.vector.tensor_copy(out=yt[:], in_=psy[:])
                    mn = asb.tile([4, 1], dt.float32, tag="mn")
                    nc.vector.reduce_sum(out=mn[:], in_=yt[:],
                                         axis=mybir.AxisListType.X)
                    nc.vector.tensor_scalar(out=mn[:], in0=mn[:],
                                            scalar1=1.0 / D, scalar2=None,
                                            op0=OP.mult)
                    xc = asb.tile([4, 128], dt.float32, tag="xc")
                    nc.vector.tensor_scalar(out=xc[:], in0=yt[:],
                                            scalar1=mn[:], scalar2=None,
                                            op0=OP.subtract)
                    sq = asb.tile([4, 128], dt.float32, tag="sq")
                    ss = asb.tile([4, 1], dt.float32, tag="ss")
                    nc.scalar.activation(sq[:], xc[:], AF.Square,
                                         accum_out=ss[:])
                    sd = asb.tile([4, 1], dt.float32, tag="sd")
                    nc.scalar.activation(sd[:], ss[:], AF.Sqrt, bias=eps_t[:],
                                         scale=1.0 / D)
                    rr = asb.tile([4, 1], dt.float32, tag="rr")
                    nc.vector.reciprocal(rr[:], sd[:])
                    yn = asb.tile([4, 128], dt.float32, tag="yn")
                    nc.vector.tensor_scalar(out=yn[:], in0=xc[:],
                                            scalar1=rr[:], scalar2=None,
                                            op0=OP.mult)
                    nc.vector.tensor_tensor(out=yn[:], in0=yn[:],
                                            in1=lng_t[:], op=OP.mult)
                    nc.vector.tensor_tensor(out=yn[:], in0=yn[:],
                                            in1=lnb_t[:], op=OP.add)
                    nc.sync.dma_start(out=y_out[:], in_=yn[:])

            def phase_stub():
                with tc.tile_pool(name="stub", bufs=1) as sp_:
                    zz = sp_.tile([GPC, D], dt.float32, name="zz")
                    nc.vector.memset(zz[:], 0)
                    p = min(phases, 3)
                    dep = (None if p == 0 else
                           g1T[0:1, 0, 0:1] if p == 1 else
                           hT2[0:1, 0, 0:1] if p == 2 else hT3[0:1, 0:1])
                    if dep is not None:
                        nc.vector.tensor_add(out=zz[0:1, 0:1], in0=dep,
                                             in1=zz[0:1, 0:1])
                    nc.sync.dma_start(out=y_out[:], in_=zz[:])

            for _ in range(repeat):
                if phases >= 1:
                    phase_l0()
                if phases >= 2:
                    phase_l2()
                if phases >= 3:
                    phase_l3()
                if phases >= 4:
                    phase_attn()
                else:
                    phase_stub()

    nc.compile()
    _prog_cache[key] = nc
    return nc


def _wrap16(arr):
    """slot i -> [i % 16, i // 16], replicated into partitions 16..31.

    CoreSim's gather ucode reads partitions 0..15; the deployed HW ucode
    reads 16..31 -- fill both so either path sees the indices.
    """
    n = arr.shape[0]
    out = np.zeros((128, n // 16), np.int16)
    w = arr.reshape(n // 16, 16).T.astype(np.int16)
    out[0:16] = w
    out[16:32] = w
    return out


def _host_prep(inputs):
    node_ids = np.asarray(inputs["node_ids"]).astype(np.int64)
    src = np.asarray(inputs["src"]).astype(np.int64)
    dst = np.asarray(inputs["dst"]).astype(np.int64)
    pad_mask = np.asarray(inputs["pad_mask"])
    ew = np.asarray(inputs["edge_weight"]).astype(np.float64)
    embed = np.asarray(inputs["embed_table"]).astype(np.float64)
    W1 = np.asarray(inputs["W1"]).astype(np.float64)
    b1 = np.asarray(inputs["b1"]).astype(np.float32)
    W2 = np.asarray(inputs["W2"]).astype(np.float32)
    b2 = np.asarray(inputs["b2"]).astype(np.float32)
    W3 = np.asarray(inputs["W3"]).astype(np.float32)
    b3 = np.asarray(inputs["b3"]).astype(np.float32)
    ipw = np.asarray(inputs["in_proj_w"]).astype(np.float64)
    ipb = np.asarray(inputs["in_proj_b"]).astype(np.float64)
    ow = np.asarray(inputs["out_w"]).astype(np.float32)
    ob = np.asarray(inputs["out_b"]).astype(np.float32)
    cls = np.asarray(inputs["cls_embedding"]).astype(np.float64).reshape(D)
    ln_g = np.asarray(inputs["ln_g"]).astype(np.float32)
    ln_b = np.asarray(inputs["ln_b"]).astype(np.float32)

    assert not pad_mask.any(), "kernel compiled for all-False pad_mask"

    # ---- shared (replicated) constants ----
    T1 = (embed @ W1).astype(BF16)                       # [1024, 256]
    Wq, Wk, Wv = ipw[:, :D], ipw[:, D:2 * D], ipw[:, 2 * D:]
    bq, bk_, bv_ = ipb[:D], ipb[D:2 * D], ipb[2 * D:]
    q_cls = (cls @ Wq + bq) / np.sqrt(HD)                # [128]
    k_cls = cls @ Wk + bk_
    v_cls = cls @ Wv                                     # no bv (added later)
    s_cls = np.array([q_cls[h * HD:(h + 1) * HD] @ k_cls[h * HD:(h + 1) * HD]
                      for h in range(H)])                # [4]
    c_h = np.array([q_cls[h * HD:(h + 1) * HD] @ bk_[h * HD:(h + 1) * HD]
                    for h in range(H)])                  # score bias fold
    e_cls = np.exp(s_cls - c_h)
    # wkq[f, h] = sum_{j in head h} Wk[f, j] * q_cls[j]
    wkq = np.zeros((128, 4), np.float64)
    for h in range(H):
        wkq[:, h] = Wk[:, h * HD:(h + 1) * HD] @ q_cls[h * HD:(h + 1) * HD]
    vc4 = np.zeros((4, 128), np.float32)
    for h in range(H):
        vc4[h, h * HD:(h + 1) * HD] = v_cls[h * HD:(h + 1) * HD]
    e4 = np.diag(e_cls).astype(np.float32)
    msel = np.zeros((128, 4), np.float32)
    for h in range(H):
        msel[h * HD:(h + 1) * HD, h] = 1.0
    # hexp[(g, h), f] = 1 iff h == head(f); gmask[(g, h), g'] = 1 iff g == g'
    hexp16 = np.zeros((16, 128), np.float32)
    gmask16 = np.zeros((16, 4), np.float32)
    for p in range(16):
        g, h = p // 4, p % 4
        hexp16[p, h * HD:(h + 1) * HD] = 1.0
        gmask16[p, g] = 1.0
    # map16[(s, t4, h), (g, h')] = 1 iff h == h' and s // 2 == g
    map16 = np.zeros((128, 16), np.float32)
    for p in range(128):
        s, t4, h = p // 16, (p % 16) // 4, p % 4
        map16[p, (s // 2) * 4 + h] = 1.0
    w2tiles = np.concatenate(
        [W2[ji * 128:(ji + 1) * 128, jo * 128:(jo + 1) * 128]
         for ji in range(2) for jo in range(2)], axis=1)  # [128, 512]
    w3tiles = np.concatenate(
        [W3[ji * 128:(ji + 1) * 128, :] for ji in range(2)], axis=1)
    shared = {
        "iota64": np.tile(np.arange(CHW, dtype=np.float32),
                          (128, 1)).astype(BF16),
        "iota512": np.tile(np.arange(SEGW, dtype=np.float32),
                           (128, 1)),
        "i128": np.eye(128, dtype=np.float32).astype(BF16),
        "i128f": np.eye(128, dtype=np.float32),
        "w2t": w2tiles.astype(BF16),
        "b2": b2.reshape(2, 128).T.copy(),
        "w3t": w3tiles.astype(BF16),
        "b3": b3.reshape(1, 128).T.copy(),
        "b1": b1.astype(np.float32).reshape(2, 128).T.copy(),
        "wkq": wkq.astype(BF16),
        "wv": Wv.astype(BF16),
        "bv": bv_.astype(np.float32).reshape(128, 1),
        "ones1": np.ones((128, 1), np.float32).astype(BF16),
        "map16": map16.astype(BF16),
        "ecls16": np.tile(e_cls.astype(np.float32), GPC).reshape(16, 1),
        "vc4": vc4.astype(BF16),
        "e4": e4.astype(BF16),
        "msel": msel,
        "hexp": hexp16,
        "gmask": gmask16,
        "eps": np.full((4, 1), 1e-5, np.float32),
        "ynb": (cls + ob).astype(np.float32).reshape(128, 1),
        "wo": ow.astype(BF16),
        "lng": np.tile(ln_g, (4, 1)),
        "lnb": np.tile(ln_b, (4, 1)),
        "t1": T1,
    }

    # ---- per-core edge partitioning ----
    ew32 = ew.astype(np.float32)
    in_maps = []
    order_all = np.argsort(dst, kind='stable')
    dst_sorted = dst[order_all]
    core_starts = np.searchsorted(dst_sorted, np.arange(0, B * N + 1, NPC))
    chunk_starts = np.searchsorted(dst_sorted, np.arange(0, B * N + 1, CHW))
    for c in range(NCORES):
        lo, hi = core_starts[c], core_starts[c + 1]
        eidx = order_all[lo:hi]
        # slot arrays: per segment [8x1024 main | 8x128 tails]
        g_idx12 = np.full(CAP, -1, np.int64)
        sl_dst = np.full(NSEG * NBAT * 128, 100.0, np.float32)
        sl_ew = np.zeros(NSEG * NBAT * 128, np.float32)
        sl3_dst = np.full(NSEG * 4 * 128, 10000.0, np.float32)
        sl3_ew = np.zeros(NSEG * 4 * 128, np.float32)
        counts = np.zeros(NCH + NSEG, np.int32)
        seg_tails = [[] for _ in range(NSEG)]
        base_chunk = c * NCH
        for k in range(NCH):
            a = chunk_starts[base_chunk + k] - lo
            bnd = chunk_starts[base_chunk + k + 1] - lo
            cnt = bnd - a
            assert cnt <= CHCAP, f"chunk overflow: {cnt} > {CHCAP}"
            e = eidx[a:bnd]
            s, kk = divmod(k, SEG)
            amain = min(cnt, ACAP)
            em, et = e[:amain], e[amain:]
            s0 = s * SEGSLOTS + kk * ACAP
            g_idx12[s0:s0 + amain] = src[em]
            seg_tails[s].append(et)
            # dst_local / ew by batch: main batches kk*8+bq
            dl = (dst[em] - (c * NPC + k * CHW)).astype(np.float32)
            we = ew32[em]
            bmain0 = s * NBAT * 128 + (kk * 8) * 128
            sl_dst[bmain0:bmain0 + amain] = dl[:amain]
            sl_ew[bmain0:bmain0 + amain] = we[:amain]
            counts[k] = max(amain, 1)
            if cnt == 0:
                g_idx12[s0] = 0
        # compact per-segment tails: overflow edges packed contiguously with
        # segment-local dst (4 batches x 128 slots, capacity 512)
        for s in range(NSEG):
            et = np.concatenate(seg_tails[s]) if seg_tails[s] else \
                np.zeros(0, np.int64)
            ntail = len(et)
            assert ntail <= 4 * 128, f"tail overflow: {ntail}"
            t0 = s * SEGSLOTS + SEG * ACAP
            g_idx12[t0:t0 + ntail] = src[et]
            if ntail == 0:
                g_idx12[t0] = 0
            b3 = s * 4 * 128
            sl3_dst[b3:b3 + ntail] = (dst[et] - (c * NPC + s * SEGW)
                                      ).astype(np.float32)
            sl3_ew[b3:b3 + ntail] = ew32[et]
            counts[NCH + s] = max(ntail, 1)
        nids_own = node_ids[c * NPC:(c + 1) * NPC]
        h0 = T1.astype(np.float32)[nids_own]          # [NPC, 256]
        h0T = h0.T.reshape(2, 128, NPC).transpose(1, 0, 2).copy()
        # layer-0 weighted count matrix C[d_local, id] = sum ew over edges
        ids_e = node_ids[src[eidx]]
        dl_e = dst[eidx] - c * NPC
        Cf = np.bincount(dl_e * N + ids_e, weights=ew[eidx],
                         minlength=NPC * N).reshape(NPC, N).astype(np.float32)
        CtT = Cf.T.astype(F8)            # [N ids, NPC]
        ct_tiles = CtT.reshape(N // 128, 128, NPC).transpose(1, 0, 2).copy()
        # remap gather indices for the half-table AllGather layout:
        # node (c', i) -> row c'*2048 + i          (i < 2048, first AG)
        #              -> 16384 + c'*2048 + i-2048 (second AG)
        v = g_idx12
        vc, vi = v // NPC, v % NPC
        g_remap = np.where(
            v < 0, v,
            np.where(vi < NPC // 2,
                     vc * (NPC // 2) + vi,
                     B * N // 2 + vc * (NPC // 2) + (vi - NPC // 2)))
        m = dict(shared)
        m.update({
            "h0w1T": h0T.astype(BF16),
            "ct": ct_tiles,
            "idx12": _wrap16(g_remap),
            "cnts": counts.reshape(1, NCH + NSEG),
            "dstl": sl_dst.reshape(NSEG * NBAT, 128).T.astype(BF16).copy(),
            "eww": sl_ew.reshape(NSEG * NBAT, 128).T.astype(BF16).copy(),
            "dstl3": sl3_dst.reshape(NSEG * 4, 128).T.copy(),
            "eww3": sl3_ew.reshape(NSEG * 4, 128).T.astype(BF16).copy(),
        })
        in_maps.append(m)
    return in_maps


def kernel(**inputs):
    from concourse.bass_utils import run_bass_kernel_spmd
    nc = _build_program()
    in_maps = _host_prep(inputs)
    res = run_bass_kernel_spmd(nc, in_maps, core_ids=list(range(NCORES)))
    y = np.concatenate([res.results[c]["y"] for c in range(NCORES)], axis=0)
    return np.ascontiguousarray(y.astype(np.float32))
